# revision 35
# baseline (speedup 1.0000x reference)
"""Trainium2 Bass kernel for nn_AttentionModule (channel self-attention).

Reference computation (per batch sample b, with x: [C=512, N=4096]):
    q   = w1 @ x + b1                     # [64, 4096]
    att = softmax(q @ q.T, axis=-1)       # [64, 64]
    out = att @ q                         # [64, 4096]
    y   = w2 @ out + b2 + x               # [512, 4096]

Sharding: data-parallel over batch. B=16 samples, 8 cores, 2 samples/core.
Small weights (w1,b1,w2,b2) replicated to every core.

Per-core design.  The kernel is HBM-bound (16.8 MB in + 16.8 MB out per
core) and the PE is power-throttled to ~1.2 GHz whenever the DMA runs
hot, so: minimum PE work, and a DMA schedule that can never stall.  Key
structural facts learned on this part (the hard way):

  - DMAHW completion-sem lanes are assigned round-robin GLOBALLY across
    both HWDGE rings, and the lane-arming wait executes on the issuing
    engine.  Any layout where a data-dependent DMA (e.g. an XBAR
    transpose of q) shares the lane space with the x-load stream couples
    loads to compute and collapses the pipeline into lockstep.  => the
    sync ring carries ONLY loads-then-stores (completion-ordered, lane
    reuse is then always benign); the ACT ring carries only the tiny
    weight loads; qT is produced by PE transposes instead of DMAs.
  - SWDGE (gpsimd) casting loads sound great (free fp32->bf16) but the
    Q7 queue is slow, blocks completion processing when it waits, and
    couples to HWDGE progress via scheduler waits — not used.

Pipeline: per-sample stream (q = w1T.T @ x f32r matmuls — f32r moving
at 512 wide is ~1.5 PE cycles/row; ACT bias-evac to bf16 q), per-row PE
transposes of q (bf16, evacuated to qT by ACT copies), Gram matmuls
deferred one row so the PE never waits on the transpose chain, softmax,
then m = (att.T @ w2T).T in ONE 512-row matmul — out = att@q is never
materialized: y = m @ q + b2 + x, with b2 and the exact fp32 x riding
the DVE evacuation (scalar_tensor_tensor computes (psum + b2) + x in
one pass).  Sample 0's step5 interleaves with sample 1's stream; s1's
q-matmuls run FIRST in each PE unit so s1's evac->transpose->gram chain
is never delayed behind step5 work.  bf16 everywhere downstream of q
(transposes, Gram, m, step5) keeps the PE at 1 cycle/row.
"""

import os
import sys
from contextlib import ExitStack

import numpy as np

for _p in ("/opt/trn_rl_repo", "/root/.axon_site/_ro/trn_rl_repo"):
    if os.path.isdir(_p) and _p not in sys.path:
        sys.path.append(_p)

import concourse.bass as bass  # noqa: E402
import concourse.tile as tile  # noqa: E402
from concourse import bacc, mybir  # noqa: E402
from concourse.bass_utils import run_bass_kernel_spmd  # noqa: E402
from concourse.masks import make_identity  # noqa: E402

F32 = mybir.dt.float32
F32R = mybir.dt.float32r
BF16 = mybir.dt.bfloat16
AF = mybir.ActivationFunctionType
ALU = mybir.AluOpType
AX = mybir.AxisListType

B, C, CR = 16, 512, 64
W, H = 64, 64
N = W * H  # 4096
NCORES = 8
BPC = B // NCORES  # samples per core
KC = C // 128  # 4 k-chunks of x / o-chunks of output
NF = 512  # moving-dim tile for the q matmuls
NN = N // NF  # 8 n-chunks
NT = N // 128  # 32 gram blocks per sample
LF = 1024  # DMA piece width (load, store)
NL = N // LF  # 4 piece rows
TPR = LF // 128  # gram blocks per piece row (8)
BPR = LF // NF  # q-matmul n-blocks per piece row (2)


def _build_nc():
    nc = bacc.Bacc(
        "TRN2",
        target_bir_lowering=False,
        debug=False,
        enable_asserts=True,
        num_devices=NCORES,
    )
    x_d = nc.dram_tensor("x", [BPC, C, N], F32, kind="ExternalInput").ap()
    w1_d = nc.dram_tensor("w1", [CR, C], F32, kind="ExternalInput").ap()
    b1_d = nc.dram_tensor("b1", [CR], F32, kind="ExternalInput").ap()
    w2_d = nc.dram_tensor("w2", [C, CR], F32, kind="ExternalInput").ap()
    b2_d = nc.dram_tensor("b2", [C], F32, kind="ExternalInput").ap()
    out_d = nc.dram_tensor("out", [BPC, C, N], F32, kind="ExternalOutput").ap()

    with tile.TileContext(nc) as tc, ExitStack() as ctx:
        singles = ctx.enter_context(tc.tile_pool(name="singles", bufs=1))
        xbf = ctx.enter_context(tc.tile_pool(name="xbf", bufs=2 * NL))
        qp = ctx.enter_context(tc.tile_pool(name="qp", bufs=2))
        qtp = ctx.enter_context(tc.tile_pool(name="qtp", bufs=2))
        mp = ctx.enter_context(tc.tile_pool(name="mp", bufs=2))
        fin = ctx.enter_context(tc.tile_pool(name="fin", bufs=8))
        small = ctx.enter_context(tc.tile_pool(name="small", bufs=2))
        ps_mm = ctx.enter_context(tc.tile_pool(name="ps_mm", bufs=2, space="PSUM"))
        ps_tp = ctx.enter_context(tc.tile_pool(name="ps_tp", bufs=2, space="PSUM"))
        ps_att = ctx.enter_context(tc.tile_pool(name="ps_att", bufs=1, space="PSUM"))
        ps_o = ctx.enter_context(tc.tile_pool(name="ps_o", bufs=3, space="PSUM"))

        # ---------- weight loads on the (otherwise DMA-free) ACT ring ------
        w1_sb = singles.tile([CR, C], F32, tag="w1")
        nc.scalar.dma_start(out=w1_sb, in_=w1_d)
        b1_sb = singles.tile([CR, 1], F32, tag="b1")
        nc.scalar.dma_start(out=b1_sb, in_=b1_d.rearrange("(c one) -> c one", one=1))
        w2cs = []
        for oc in range(KC):
            w2c = small.tile([128, CR], F32, tag="w2chunk", name=f"w2c{oc}")
            nc.scalar.dma_start(out=w2c, in_=w2_d[oc * 128 : (oc + 1) * 128, :])
            w2cs.append(w2c)
        b2cs = []
        for oc in range(KC):
            b2c = singles.tile([128, 1], F32, tag=f"b2c{oc}")
            nc.scalar.dma_start(
                out=b2c,
                in_=b2_d[oc * 128 : (oc + 1) * 128].rearrange(
                    "(p one) -> p one", one=1
                ),
            )
            b2cs.append(b2c)

        # ---------- x loads: per-piece f32r tiles on the sync ring ----------
        xb = [[[None] * KC for _ in range(NL)] for _ in range(BPC)]

        def load_x_rows(s, rows):
            for j in rows:
                lsl = bass.ts(j, LF)
                for k in range(KC):
                    t = xbf.tile(
                        [128, LF], F32R, tag=f"xb{k}", name=f"xb{s}_{j}_{k}"
                    )
                    nc.sync.dma_start(
                        out=t,
                        in_=x_d[s, k * 128 : (k + 1) * 128, lsl].bitcast(F32R),
                    )
                    xb[s][j][k] = t

        load_x_rows(0, [0, 1])

        ident = singles.tile([128, 128], F32, tag="ident")
        make_identity(nc, ident)
        identB = singles.tile([128, 128], BF16, tag="identB")
        nc.gpsimd.tensor_copy(identB, ident)

        # ---------- weight prep (PE transposes via the att psum ring) -----
        w1T = singles.tile([128, KC, CR], F32R, tag="w1T")
        for k in range(KC):
            ptp = ps_att.tile([128, CR], F32, tag="att", name=f"w1tp{k}")
            nc.tensor.transpose(
                ptp, w1_sb[:, k * 128 : (k + 1) * 128], ident[0:CR, 0:CR]
            )
            nc.vector.tensor_copy(w1T[:, k, :], ptp)
        w2T = singles.tile([CR, C], BF16, tag="w2T")
        for oc in range(KC):
            ptp = ps_att.tile([CR, 128], F32, tag="att", name=f"w2tp{oc}")
            nc.tensor.transpose(ptp, w2cs[oc], ident)
            nc.vector.tensor_copy(w2T[:, oc * 128 : (oc + 1) * 128], ptp)

        # ---------- per-sample phases ----------
        state = {}

        def begin_sample(s):
            state[s] = {
                "q": qp.tile([CR, N], BF16, tag="q", name=f"q{s}"),
                "qT": qtp.tile([128, NT, CR], BF16, tag="qT", name=f"qT{s}"),
                "patt": ps_att.tile([CR, CR], F32, tag="att", name=f"att{s}"),
                "m": mp.tile([CR, C], BF16, tag="m", name=f"m{s}"),
            }

        def stream_row(s, j):
            """q matmuls + ACT bias-evacuation for piece row j."""
            st = state[s]
            q = st["q"]
            for h in range(BPR):
                n = j * BPR + h
                nsl = bass.ts(n, NF)
                hsl = bass.ts(h, NF)
                pq = ps_mm.tile([CR, NF], F32, tag="mm", name=f"pq{s}_{n}")
                for k in range(KC):
                    nc.tensor.matmul(
                        pq, w1T[:, k, :], xb[s][j][k][:, hsl],
                        start=(k == 0), stop=(k == KC - 1),
                    )
                nc.scalar.activation(
                    q[:, nsl], pq, AF.Identity, bias=b1_sb, scale=1.0
                )

        def tp_row(s, j):
            """PE transposes of q row j, evacuated to qT by ACT copies.

            DMA-based transposes were tried and abandoned: the 8 DMAHW
            completion-sem lanes are shared globally, so a transpose DMA's
            lane inevitably anchors some later x-load, and its own
            data-dependency (the q evacs) then stalls the whole load ring."""
            st = state[s]
            q, qT = st["q"], st["qT"]
            for t_i in range(TPR * j, TPR * (j + 1)):
                ptp = ps_tp.tile([128, CR], BF16, tag="tp", name=f"tp{s}_{t_i}")
                nc.tensor.transpose(
                    ptp, q[:, t_i * 128 : (t_i + 1) * 128], identB[0:CR, 0:CR]
                )
                nc.scalar.copy(qT[:, t_i, :], ptp)

        def gram_row(s, j):
            st = state[s]
            qT, patt = st["qT"], st["patt"]
            for t_i in range(TPR * j, TPR * (j + 1)):
                qTs = qT[:, t_i, :]
                nc.tensor.matmul(
                    patt, qTs, qTs, start=(t_i == 0), stop=(t_i == NT - 1)
                )

        def softmax_m(s):
            st = state[s]
            patt, m = st["patt"], st["m"]
            negm = small.tile([CR, 1], F32, tag="negm", name=f"negm{s}")
            nc.vector.tensor_reduce(
                out=negm, in_=patt, axis=AX.X, op=ALU.max, negate=True
            )
            shifted = small.tile([CR, CR], F32, tag="shifted", name=f"shifted{s}")
            nc.vector.tensor_scalar(
                out=shifted, in0=patt, scalar1=negm, scalar2=-80.0,
                op0=ALU.add, op1=ALU.max,
            )
            atte = small.tile([CR, CR], F32, tag="atte", name=f"atte{s}")
            ssum = small.tile([CR, 1], F32, tag="ssum", name=f"ssum{s}")
            nc.scalar.activation(
                atte, shifted, AF.Exp, bias=0.0, scale=1.0, accum_out=ssum
            )
            rsum = small.tile([CR, 1], F32, tag="rsum", name=f"rsum{s}")
            nc.vector.reciprocal(rsum, ssum)
            attn = small.tile([CR, CR], BF16, tag="attn", name=f"attn{s}")
            nc.vector.tensor_scalar_mul(attn, atte, rsum)
            # m rows = (att.T @ w2T) = (w2 @ att).T in one 512-row matmul
            pmT = ps_mm.tile([CR, C], F32, tag="mm", name=f"pmT{s}")
            nc.tensor.matmul(pmT, attn, w2T, start=True, stop=True)
            nc.scalar.copy(m, pmT)

        def step5_chunk(s, oc):
            """y[oc] = m[oc] @ q + b2[oc] + x[oc] into fin tiles."""
            st = state[s]
            q, m = st["q"], st["m"]
            osl = slice(oc * 128, (oc + 1) * 128)
            fins = []
            for half in range(NL):
                f = fin.tile([128, LF], F32, tag="fin", name=f"fin{s}_{oc}_{half}")
                for sub in range(BPR):
                    n = half * BPR + sub
                    nsl = bass.ts(n, NF)
                    ssl = bass.ts(sub, NF)
                    p5 = ps_o.tile([128, NF], F32, tag="o5", name=f"p5{s}_{oc}_{n}")
                    nc.tensor.matmul(
                        p5, m[:, osl], q[:, nsl], start=True, stop=True
                    )
                    nc.vector.scalar_tensor_tensor(
                        out=f[:, ssl], in0=p5, scalar=b2cs[oc],
                        in1=xb[s][half][oc][:, ssl].bitcast(F32),
                        op0=ALU.add, op1=ALU.add,
                    )
                fins.append((s, oc, half, f))
            return fins

        def issue_stores(fins):
            for s, oc, half, f in fins:
                osl = slice(oc * 128, (oc + 1) * 128)
                nc.sync.dma_start(out=out_d[s, osl, bass.ts(half, LF)], in_=f)

        # ================= sample 0 stream =================
        # sync ring holds ONLY loads (all 32, completion-ordered) and then
        # stores; qT transposes ride the ACT ring right behind their evacs.
        load_x_rows(0, [2, 3])
        load_x_rows(1, [0, 1, 2, 3])
        begin_sample(0)
        stream_row(0, 0)
        tp_row(0, 0)
        stream_row(0, 1)
        tp_row(0, 1)
        gram_row(0, 0)
        stream_row(0, 2)
        tp_row(0, 2)
        gram_row(0, 1)
        stream_row(0, 3)
        tp_row(0, 3)
        gram_row(0, 2)
        gram_row(0, 3)
        softmax_m(0)

        # ========== interleave: s1 stream first-in-unit, s0 step5 ==========
        begin_sample(1)
        fins = []
        for i in range(KC):
            stream_row(1, i)
            tp_row(1, i)
            if i > 0:
                gram_row(1, i - 1)
            fins += step5_chunk(0, i)
            issue_stores(fins[-NL:])
        gram_row(1, NL - 1)
        softmax_m(1)
        for i in range(KC):
            fins1 = step5_chunk(1, i)
            issue_stores(fins1)

    nc.compile()
    return nc


_NC_CACHE = None


def _get_nc():
    global _NC_CACHE
    if _NC_CACHE is None:
        _NC_CACHE = _build_nc()
    return _NC_CACHE


def _as_f32(a):
    return np.ascontiguousarray(np.asarray(a, dtype=np.float32))


def run(inputs, trace=False):
    """Run on all 8 cores; returns (full output [B,C,W,H], BassKernelResults)."""
    nc = _get_nc()
    x = _as_f32(inputs["x"]).reshape(B, C, N)
    w1 = _as_f32(inputs["w1"])
    b1 = _as_f32(inputs["b1"])
    w2 = _as_f32(inputs["w2"])
    b2 = _as_f32(inputs["b2"])
    in_maps = [
        {
            "x": x[c * BPC : (c + 1) * BPC],
            "w1": w1,
            "b1": b1,
            "w2": w2,
            "b2": b2,
        }
        for c in range(NCORES)
    ]
    res = run_bass_kernel_spmd(nc, in_maps, list(range(NCORES)), trace=trace)
    out = np.concatenate([res.results[c]["out"] for c in range(NCORES)], axis=0)
    return out.reshape(B, C, W, H).astype(np.float32, copy=False), res


def kernel(**inputs):
    out, _ = run(inputs)
    return out


# revision 36
# speedup vs baseline: 1.0109x; 1.0109x over previous
"""Trainium2 Bass kernel for nn_AttentionModule (channel self-attention).

Reference computation (per batch sample b, with x: [C=512, N=4096]):
    q   = w1 @ x + b1                     # [64, 4096]
    att = softmax(q @ q.T, axis=-1)       # [64, 64]
    out = att @ q                         # [64, 4096]
    y   = w2 @ out + b2 + x               # [512, 4096]

Sharding: data-parallel over batch. B=16 samples, 8 cores, 2 samples/core.
Small weights (w1,b1,w2,b2) replicated to every core.

Per-core design.  The kernel is HBM-bound (16.8 MB in + 16.8 MB out per
core) and the PE is power-throttled to ~1.2 GHz whenever the DMA runs
hot, so: minimum PE work, and a DMA schedule that can never stall.  Key
structural facts learned on this part (the hard way):

  - DMAHW completion-sem lanes are assigned round-robin GLOBALLY across
    both HWDGE rings, and the lane-arming wait executes on the issuing
    engine.  Any layout where a data-dependent DMA (e.g. an XBAR
    transpose of q) shares the lane space with the x-load stream couples
    loads to compute and collapses the pipeline into lockstep.  => the
    sync ring carries ONLY loads-then-stores (completion-ordered, lane
    reuse is then always benign); the ACT ring carries only the tiny
    weight loads; qT is produced by PE transposes instead of DMAs.
  - SWDGE (gpsimd) casting loads sound great (free fp32->bf16) but the
    Q7 queue is slow, blocks completion processing when it waits, and
    couples to HWDGE progress via scheduler waits — not used.

Pipeline: per-sample stream (q = w1T.T @ x f32r matmuls — f32r moving
at 512 wide is ~1.5 PE cycles/row; ACT bias-evac to bf16 q), per-row PE
transposes of q (bf16, evacuated to qT by ACT copies), Gram matmuls
deferred one row so the PE never waits on the transpose chain, softmax,
then m = (att.T @ w2T).T in ONE 512-row matmul — out = att@q is never
materialized: y = m @ q + b2 + x, with b2 and the exact fp32 x riding
the DVE evacuation (scalar_tensor_tensor computes (psum + b2) + x in
one pass).  Sample 0's step5 interleaves with sample 1's stream; s1's
q-matmuls run FIRST in each PE unit so s1's evac->transpose->gram chain
is never delayed behind step5 work.  bf16 everywhere downstream of q
(transposes, Gram, m, step5) keeps the PE at 1 cycle/row.
"""

import os
import sys
from contextlib import ExitStack

import numpy as np

for _p in ("/opt/trn_rl_repo", "/root/.axon_site/_ro/trn_rl_repo"):
    if os.path.isdir(_p) and _p not in sys.path:
        sys.path.append(_p)

import concourse.bass as bass  # noqa: E402
import concourse.tile as tile  # noqa: E402
from concourse import bacc, mybir  # noqa: E402
from concourse.bass_utils import run_bass_kernel_spmd  # noqa: E402
from concourse.masks import make_identity  # noqa: E402

F32 = mybir.dt.float32
F32R = mybir.dt.float32r
BF16 = mybir.dt.bfloat16
AF = mybir.ActivationFunctionType
ALU = mybir.AluOpType
AX = mybir.AxisListType

B, C, CR = 16, 512, 64
W, H = 64, 64
N = W * H  # 4096
NCORES = 8
BPC = B // NCORES  # samples per core
KC = C // 128  # 4 k-chunks of x / o-chunks of output
NF = 512  # moving-dim tile for the q matmuls
NN = N // NF  # 8 n-chunks
NT = N // 128  # 32 gram blocks per sample
LF = 1024  # DMA piece width (load, store)
NL = N // LF  # 4 piece rows
TPR = LF // 128  # gram blocks per piece row (8)
BPR = LF // NF  # q-matmul n-blocks per piece row (2)


def _build_nc():
    nc = bacc.Bacc(
        "TRN2",
        target_bir_lowering=False,
        debug=False,
        enable_asserts=True,
        num_devices=NCORES,
    )
    x_d = nc.dram_tensor("x", [BPC, C, N], F32, kind="ExternalInput").ap()
    w1_d = nc.dram_tensor("w1", [CR, C], F32, kind="ExternalInput").ap()
    b1_d = nc.dram_tensor("b1", [CR], F32, kind="ExternalInput").ap()
    w2_d = nc.dram_tensor("w2", [C, CR], F32, kind="ExternalInput").ap()
    b2_d = nc.dram_tensor("b2", [C], F32, kind="ExternalInput").ap()
    out_d = nc.dram_tensor("out", [BPC, C, N], F32, kind="ExternalOutput").ap()

    with tile.TileContext(nc) as tc, ExitStack() as ctx:
        singles = ctx.enter_context(tc.tile_pool(name="singles", bufs=1))
        xbf = ctx.enter_context(tc.tile_pool(name="xbf", bufs=2 * NL))
        qp = ctx.enter_context(tc.tile_pool(name="qp", bufs=2))
        qtp = ctx.enter_context(tc.tile_pool(name="qtp", bufs=2))
        mp = ctx.enter_context(tc.tile_pool(name="mp", bufs=2))
        fin = ctx.enter_context(tc.tile_pool(name="fin", bufs=8))
        small = ctx.enter_context(tc.tile_pool(name="small", bufs=2))
        ps_mm = ctx.enter_context(tc.tile_pool(name="ps_mm", bufs=2, space="PSUM"))
        ps_tp = ctx.enter_context(tc.tile_pool(name="ps_tp", bufs=2, space="PSUM"))
        ps_att = ctx.enter_context(tc.tile_pool(name="ps_att", bufs=1, space="PSUM"))
        ps_o = ctx.enter_context(tc.tile_pool(name="ps_o", bufs=3, space="PSUM"))

        # ---------- weight loads on the (otherwise DMA-free) ACT ring ------
        w1_sb = singles.tile([CR, C], F32, tag="w1")
        nc.scalar.dma_start(out=w1_sb, in_=w1_d)
        b1_sb = singles.tile([CR, 1], F32, tag="b1")
        nc.scalar.dma_start(out=b1_sb, in_=b1_d.rearrange("(c one) -> c one", one=1))
        w2cs = []
        for oc in range(KC):
            w2c = small.tile([128, CR], F32, tag="w2chunk", name=f"w2c{oc}")
            nc.scalar.dma_start(out=w2c, in_=w2_d[oc * 128 : (oc + 1) * 128, :])
            w2cs.append(w2c)
        b2cs = []
        for oc in range(KC):
            b2c = singles.tile([128, 1], F32, tag=f"b2c{oc}")
            nc.scalar.dma_start(
                out=b2c,
                in_=b2_d[oc * 128 : (oc + 1) * 128].rearrange(
                    "(p one) -> p one", one=1
                ),
            )
            b2cs.append(b2c)

        # ---------- x loads: per-piece f32r tiles on the sync ring ----------
        xb = [[[None] * KC for _ in range(NL)] for _ in range(BPC)]

        def load_x_rows(s, rows):
            for j in rows:
                lsl = bass.ts(j, LF)
                for k in range(KC):
                    t = xbf.tile(
                        [128, LF], F32R, tag=f"xb{k}", name=f"xb{s}_{j}_{k}"
                    )
                    nc.sync.dma_start(
                        out=t,
                        in_=x_d[s, k * 128 : (k + 1) * 128, lsl].bitcast(F32R),
                    )
                    xb[s][j][k] = t

        load_x_rows(0, [0, 1])

        ident = singles.tile([128, 128], F32, tag="ident")
        make_identity(nc, ident)
        identB = singles.tile([128, 128], BF16, tag="identB")
        nc.gpsimd.tensor_copy(identB, ident)

        # ---------- weight prep (PE transposes via the att psum ring) -----
        w1T = singles.tile([128, KC, CR], F32R, tag="w1T")
        for k in range(KC):
            ptp = ps_att.tile([128, CR], F32, tag="att", name=f"w1tp{k}")
            nc.tensor.transpose(
                ptp, w1_sb[:, k * 128 : (k + 1) * 128], ident[0:CR, 0:CR]
            )
            nc.vector.tensor_copy(w1T[:, k, :], ptp)
        w2T = singles.tile([CR, C], BF16, tag="w2T")
        for oc in range(KC):
            ptp = ps_att.tile([CR, 128], F32, tag="att", name=f"w2tp{oc}")
            nc.tensor.transpose(ptp, w2cs[oc], ident)
            nc.vector.tensor_copy(w2T[:, oc * 128 : (oc + 1) * 128], ptp)

        # ---------- per-sample phases ----------
        state = {}

        def begin_sample(s):
            state[s] = {
                "q": qp.tile([CR, N], BF16, tag="q", name=f"q{s}"),
                "qT": qtp.tile([128, NT, CR], BF16, tag="qT", name=f"qT{s}"),
                "patt": ps_att.tile([CR, CR], F32, tag="att", name=f"att{s}"),
                "m": mp.tile([CR, C], BF16, tag="m", name=f"m{s}"),
            }

        def stream_row(s, j):
            """q matmuls + ACT bias-evacuation for piece row j."""
            st = state[s]
            q = st["q"]
            for h in range(BPR):
                n = j * BPR + h
                nsl = bass.ts(n, NF)
                hsl = bass.ts(h, NF)
                pq = ps_mm.tile([CR, NF], F32, tag="mm", name=f"pq{s}_{n}")
                for k in range(KC):
                    nc.tensor.matmul(
                        pq, w1T[:, k, :], xb[s][j][k][:, hsl],
                        start=(k == 0), stop=(k == KC - 1),
                    )
                nc.scalar.activation(
                    q[:, nsl], pq, AF.Identity, bias=b1_sb, scale=1.0
                )

        def tp_row(s, j):
            """PE transposes of q row j, evacuated to qT by ACT copies.

            DMA-based transposes were tried and abandoned: the 8 DMAHW
            completion-sem lanes are shared globally, so a transpose DMA's
            lane inevitably anchors some later x-load, and its own
            data-dependency (the q evacs) then stalls the whole load ring."""
            st = state[s]
            q, qT = st["q"], st["qT"]
            # two transposes per PSUM tile + one ACT copy per pair: halves
            # the PE<->ACT ping-pong depth that paces this chain
            for p in range(TPR // 2):
                t0 = TPR * j + 2 * p
                ptp = ps_tp.tile([128, 2, CR], BF16, tag="tp", name=f"tp{s}_{t0}")
                for u in range(2):
                    t_i = t0 + u
                    nc.tensor.transpose(
                        ptp[:, u, :],
                        q[:, t_i * 128 : (t_i + 1) * 128],
                        identB[0:CR, 0:CR],
                    )
                nc.scalar.copy(qT[:, t0 : t0 + 2, :], ptp)

        def gram_row(s, j):
            st = state[s]
            qT, patt = st["qT"], st["patt"]
            for t_i in range(TPR * j, TPR * (j + 1)):
                qTs = qT[:, t_i, :]
                nc.tensor.matmul(
                    patt, qTs, qTs, start=(t_i == 0), stop=(t_i == NT - 1)
                )

        def softmax_m(s):
            st = state[s]
            patt, m = st["patt"], st["m"]
            negm = small.tile([CR, 1], F32, tag="negm", name=f"negm{s}")
            nc.vector.tensor_reduce(
                out=negm, in_=patt, axis=AX.X, op=ALU.max, negate=True
            )
            shifted = small.tile([CR, CR], F32, tag="shifted", name=f"shifted{s}")
            nc.vector.tensor_scalar(
                out=shifted, in0=patt, scalar1=negm, scalar2=-80.0,
                op0=ALU.add, op1=ALU.max,
            )
            atte = small.tile([CR, CR], F32, tag="atte", name=f"atte{s}")
            ssum = small.tile([CR, 1], F32, tag="ssum", name=f"ssum{s}")
            nc.scalar.activation(
                atte, shifted, AF.Exp, bias=0.0, scale=1.0, accum_out=ssum
            )
            rsum = small.tile([CR, 1], F32, tag="rsum", name=f"rsum{s}")
            nc.vector.reciprocal(rsum, ssum)
            attn = small.tile([CR, CR], BF16, tag="attn", name=f"attn{s}")
            nc.vector.tensor_scalar_mul(attn, atte, rsum)
            # m rows = (att.T @ w2T) = (w2 @ att).T in one 512-row matmul
            pmT = ps_mm.tile([CR, C], F32, tag="mm", name=f"pmT{s}")
            nc.tensor.matmul(pmT, attn, w2T, start=True, stop=True)
            nc.scalar.copy(m, pmT)

        def step5_chunk(s, oc):
            """y[oc] = m[oc] @ q + b2[oc] + x[oc] into fin tiles."""
            st = state[s]
            q, m = st["q"], st["m"]
            osl = slice(oc * 128, (oc + 1) * 128)
            fins = []
            for half in range(NL):
                f = fin.tile([128, LF], F32, tag="fin", name=f"fin{s}_{oc}_{half}")
                for sub in range(BPR):
                    n = half * BPR + sub
                    nsl = bass.ts(n, NF)
                    ssl = bass.ts(sub, NF)
                    p5 = ps_o.tile([128, NF], F32, tag="o5", name=f"p5{s}_{oc}_{n}")
                    nc.tensor.matmul(
                        p5, m[:, osl], q[:, nsl], start=True, stop=True
                    )
                    nc.vector.scalar_tensor_tensor(
                        out=f[:, ssl], in0=p5, scalar=b2cs[oc],
                        in1=xb[s][half][oc][:, ssl].bitcast(F32),
                        op0=ALU.add, op1=ALU.add,
                    )
                fins.append((s, oc, half, f))
            return fins

        def issue_stores(fins):
            for s, oc, half, f in fins:
                osl = slice(oc * 128, (oc + 1) * 128)
                nc.sync.dma_start(out=out_d[s, osl, bass.ts(half, LF)], in_=f)

        # ================= sample 0 stream =================
        # sync ring holds ONLY loads (all 32, completion-ordered) and then
        # stores; qT transposes ride the ACT ring right behind their evacs.
        load_x_rows(0, [2, 3])
        load_x_rows(1, [0, 1, 2, 3])
        begin_sample(0)
        stream_row(0, 0)
        tp_row(0, 0)
        stream_row(0, 1)
        tp_row(0, 1)
        gram_row(0, 0)
        stream_row(0, 2)
        tp_row(0, 2)
        gram_row(0, 1)
        stream_row(0, 3)
        tp_row(0, 3)
        gram_row(0, 2)
        gram_row(0, 3)
        softmax_m(0)

        # ========== interleave: s1 stream first-in-unit, s0 step5 ==========
        begin_sample(1)
        fins = []
        for i in range(KC):
            stream_row(1, i)
            tp_row(1, i)
            if i > 0:
                gram_row(1, i - 1)
            fins += step5_chunk(0, i)
            issue_stores(fins[-NL:])
        gram_row(1, NL - 1)
        softmax_m(1)
        for i in range(KC):
            fins1 = step5_chunk(1, i)
            issue_stores(fins1)

    nc.compile()
    return nc


_NC_CACHE = None


def _get_nc():
    global _NC_CACHE
    if _NC_CACHE is None:
        _NC_CACHE = _build_nc()
    return _NC_CACHE


def _as_f32(a):
    return np.ascontiguousarray(np.asarray(a, dtype=np.float32))


def run(inputs, trace=False):
    """Run on all 8 cores; returns (full output [B,C,W,H], BassKernelResults)."""
    nc = _get_nc()
    x = _as_f32(inputs["x"]).reshape(B, C, N)
    w1 = _as_f32(inputs["w1"])
    b1 = _as_f32(inputs["b1"])
    w2 = _as_f32(inputs["w2"])
    b2 = _as_f32(inputs["b2"])
    in_maps = [
        {
            "x": x[c * BPC : (c + 1) * BPC],
            "w1": w1,
            "b1": b1,
            "w2": w2,
            "b2": b2,
        }
        for c in range(NCORES)
    ]
    res = run_bass_kernel_spmd(nc, in_maps, list(range(NCORES)), trace=trace)
    out = np.concatenate([res.results[c]["out"] for c in range(NCORES)], axis=0)
    return out.reshape(B, C, W, H).astype(np.float32, copy=False), res


def kernel(**inputs):
    out, _ = run(inputs)
    return out


# revision 41
# speedup vs baseline: 1.0269x; 1.0159x over previous
"""Trainium2 Bass kernel for nn_AttentionModule (channel self-attention).

Reference computation (per batch sample b, with x: [C=512, N=4096]):
    q   = w1 @ x + b1                     # [64, 4096]
    att = softmax(q @ q.T, axis=-1)       # [64, 64]
    out = att @ q                         # [64, 4096]
    y   = w2 @ out + b2 + x               # [512, 4096]

Sharding: data-parallel over batch. B=16 samples, 8 cores, 2 samples/core.
Small weights (w1,b1,w2,b2) replicated to every core.

Per-core design.  The kernel is HBM-bound (16.8 MB in + 16.8 MB out per
core) and the PE is power-throttled to ~1.2 GHz whenever the DMA runs
hot, so: minimum PE work, and a DMA schedule that can never stall.  Key
structural facts learned on this part (the hard way):

  - DMAHW completion-sem lanes are assigned round-robin GLOBALLY across
    both HWDGE rings, and the lane-arming wait executes on the issuing
    engine.  Any layout where a data-dependent DMA (e.g. an XBAR
    transpose of q) shares the lane space with the x-load stream couples
    loads to compute and collapses the pipeline into lockstep.  => the
    sync ring carries ONLY loads-then-stores (completion-ordered, lane
    reuse is then always benign); the ACT ring carries only the tiny
    weight loads; qT is produced by PE transposes instead of DMAs.
  - SWDGE (gpsimd) casting loads sound great (free fp32->bf16) but the
    Q7 queue is slow, blocks completion processing when it waits, and
    couples to HWDGE progress via scheduler waits — not used.

Pipeline: per-sample stream (q = w1T.T @ x f32r matmuls — f32r moving
at 512 wide is ~1.5 PE cycles/row; ACT bias-evac to bf16 q), per-row PE
transposes of q (bf16, evacuated to qT by ACT copies), Gram matmuls
deferred one row so the PE never waits on the transpose chain, softmax,
then m = (att.T @ w2T).T in ONE 512-row matmul — out = att@q is never
materialized: y = m @ q + b2 + x, with b2 and the exact fp32 x riding
the DVE evacuation (scalar_tensor_tensor computes (psum + b2) + x in
one pass).  Sample 0's step5 interleaves with sample 1's stream; s1's
q-matmuls run FIRST in each PE unit so s1's evac->transpose->gram chain
is never delayed behind step5 work.  bf16 everywhere downstream of q
(transposes, Gram, m, step5) keeps the PE at 1 cycle/row.
"""

import os
import sys
from contextlib import ExitStack

import numpy as np

for _p in ("/opt/trn_rl_repo", "/root/.axon_site/_ro/trn_rl_repo"):
    if os.path.isdir(_p) and _p not in sys.path:
        sys.path.append(_p)

import concourse.bass as bass  # noqa: E402
import concourse.tile as tile  # noqa: E402
from concourse import bacc, mybir  # noqa: E402
from concourse.bass_utils import run_bass_kernel_spmd  # noqa: E402
from concourse.masks import make_identity  # noqa: E402

F32 = mybir.dt.float32
F32R = mybir.dt.float32r
BF16 = mybir.dt.bfloat16
AF = mybir.ActivationFunctionType
ALU = mybir.AluOpType
AX = mybir.AxisListType

B, C, CR = 16, 512, 64
W, H = 64, 64
N = W * H  # 4096
NCORES = 8
BPC = B // NCORES  # samples per core
KC = C // 128  # 4 k-chunks of x / o-chunks of output
NF = 512  # moving-dim tile for the q matmuls
NN = N // NF  # 8 n-chunks
NT = N // 128  # 32 gram blocks per sample
LF = 1024  # DMA piece width (load, store)
NL = N // LF  # 4 piece rows
TPR = LF // 128  # gram blocks per piece row (8)
BPR = LF // NF  # q-matmul n-blocks per piece row (2)


def _build_nc():
    nc = bacc.Bacc(
        "TRN2",
        target_bir_lowering=False,
        debug=False,
        enable_asserts=True,
        num_devices=NCORES,
    )
    x_d = nc.dram_tensor("x", [BPC, C, N], F32, kind="ExternalInput").ap()
    w1_d = nc.dram_tensor("w1", [CR, C], F32, kind="ExternalInput").ap()
    b1_d = nc.dram_tensor("b1", [CR], F32, kind="ExternalInput").ap()
    w2_d = nc.dram_tensor("w2", [C, CR], F32, kind="ExternalInput").ap()
    b2_d = nc.dram_tensor("b2", [C], F32, kind="ExternalInput").ap()
    out_d = nc.dram_tensor("out", [BPC, C, N], F32, kind="ExternalOutput").ap()

    with tile.TileContext(nc) as tc, ExitStack() as ctx:
        singles = ctx.enter_context(tc.tile_pool(name="singles", bufs=1))
        xstg = ctx.enter_context(tc.tile_pool(name="xstg", bufs=3))
        xbf = ctx.enter_context(tc.tile_pool(name="xbf", bufs=2 * NL))
        qp = ctx.enter_context(tc.tile_pool(name="qp", bufs=2))
        qtp = ctx.enter_context(tc.tile_pool(name="qtp", bufs=2))
        mp = ctx.enter_context(tc.tile_pool(name="mp", bufs=2))
        fin = ctx.enter_context(tc.tile_pool(name="fin", bufs=8))
        small = ctx.enter_context(tc.tile_pool(name="small", bufs=2))
        ps_mm = ctx.enter_context(tc.tile_pool(name="ps_mm", bufs=2, space="PSUM"))
        ps_tp = ctx.enter_context(tc.tile_pool(name="ps_tp", bufs=2, space="PSUM"))
        ps_att = ctx.enter_context(tc.tile_pool(name="ps_att", bufs=1, space="PSUM"))
        ps_o = ctx.enter_context(tc.tile_pool(name="ps_o", bufs=3, space="PSUM"))

        # ---------- weight loads on the (otherwise DMA-free) ACT ring ------
        w1_sb = singles.tile([CR, C], F32, tag="w1")
        nc.scalar.dma_start(out=w1_sb, in_=w1_d)
        b1_sb = singles.tile([CR, 1], F32, tag="b1")
        nc.scalar.dma_start(out=b1_sb, in_=b1_d.rearrange("(c one) -> c one", one=1))
        w2cs = []
        for oc in range(KC):
            w2c = small.tile([128, CR], F32, tag="w2chunk", name=f"w2c{oc}")
            nc.scalar.dma_start(out=w2c, in_=w2_d[oc * 128 : (oc + 1) * 128, :])
            w2cs.append(w2c)
        b2cs = []
        for oc in range(KC):
            b2c = singles.tile([128, 1], F32, tag=f"b2c{oc}")
            nc.scalar.dma_start(
                out=b2c,
                in_=b2_d[oc * 128 : (oc + 1) * 128].rearrange(
                    "(p one) -> p one", one=1
                ),
            )
            b2cs.append(b2c)

        # ---------- x loads: fp32 staging tiles on the sync ring, cast to
        # persistent per-piece bf16 tiles by ACT (k0,k1) / DVE (k2,k3) —
        # both engines have large slack vs the 5.8us row cadence ----------
        xsg = [[[None] * KC for _ in range(NL)] for _ in range(BPC)]
        xb = [[[None] * KC for _ in range(NL)] for _ in range(BPC)]

        def load_x_rows(s, rows):
            for j in rows:
                lsl = bass.ts(j, LF)
                for k in range(KC):
                    t = xstg.tile(
                        [128, LF], F32, tag=f"st{k}", name=f"st{s}_{j}_{k}"
                    )
                    nc.sync.dma_start(
                        out=t, in_=x_d[s, k * 128 : (k + 1) * 128, lsl]
                    )
                    xsg[s][j][k] = t

        def cast_row(s, j):
            for k in range(KC):
                t = xbf.tile([128, LF], BF16, tag=f"xb{k}", name=f"xb{s}_{j}_{k}")
                if k < 2:
                    nc.scalar.copy(t, xsg[s][j][k])
                else:
                    nc.vector.tensor_copy(t, xsg[s][j][k])
                xb[s][j][k] = t

        load_x_rows(0, [0, 1])

        ident = singles.tile([128, 128], F32, tag="ident")
        make_identity(nc, ident)
        identB = singles.tile([128, 128], BF16, tag="identB")
        nc.gpsimd.tensor_copy(identB, ident)

        # ---------- weight prep (PE transposes via the att psum ring) -----
        w1T = singles.tile([128, KC, CR], BF16, tag="w1T")
        for k in range(KC):
            ptp = ps_att.tile([128, CR], F32, tag="att", name=f"w1tp{k}")
            nc.tensor.transpose(
                ptp, w1_sb[:, k * 128 : (k + 1) * 128], ident[0:CR, 0:CR]
            )
            nc.vector.tensor_copy(w1T[:, k, :], ptp)
        w2T = singles.tile([CR, C], BF16, tag="w2T")
        for oc in range(KC):
            ptp = ps_att.tile([CR, 128], F32, tag="att", name=f"w2tp{oc}")
            nc.tensor.transpose(ptp, w2cs[oc], ident)
            nc.vector.tensor_copy(w2T[:, oc * 128 : (oc + 1) * 128], ptp)

        # ---------- per-sample phases ----------
        state = {}

        def begin_sample(s):
            state[s] = {
                "q": qp.tile([CR, N], BF16, tag="q", name=f"q{s}"),
                "qT": qtp.tile([128, NT, CR], BF16, tag="qT", name=f"qT{s}"),
                "patt": ps_att.tile([CR, CR], F32, tag="att", name=f"att{s}"),
                "m": mp.tile([CR, C], BF16, tag="m", name=f"m{s}"),
            }

        def stream_row(s, j):
            """casts + q matmuls + ACT bias-evacuation for piece row j."""
            cast_row(s, j)
            st = state[s]
            q = st["q"]
            for h in range(BPR):
                n = j * BPR + h
                nsl = bass.ts(n, NF)
                hsl = bass.ts(h, NF)
                pq = ps_mm.tile([CR, NF], F32, tag="mm", name=f"pq{s}_{n}")
                for k in range(KC):
                    nc.tensor.matmul(
                        pq, w1T[:, k, :], xb[s][j][k][:, hsl],
                        start=(k == 0), stop=(k == KC - 1),
                    )
                nc.scalar.activation(
                    q[:, nsl], pq, AF.Identity, bias=b1_sb, scale=1.0
                )

        def tp_row(s, j):
            """PE transposes of q row j, evacuated to qT by ACT copies.

            DMA-based transposes were tried and abandoned: the 8 DMAHW
            completion-sem lanes are shared globally, so a transpose DMA's
            lane inevitably anchors some later x-load, and its own
            data-dependency (the q evacs) then stalls the whole load ring."""
            st = state[s]
            q, qT = st["q"], st["qT"]
            # two transposes per PSUM tile + one ACT copy per pair: halves
            # the PE<->ACT ping-pong depth that paces this chain
            for p in range(TPR // 2):
                t0 = TPR * j + 2 * p
                ptp = ps_tp.tile([128, 2, CR], BF16, tag="tp", name=f"tp{s}_{t0}")
                for u in range(2):
                    t_i = t0 + u
                    nc.tensor.transpose(
                        ptp[:, u, :],
                        q[:, t_i * 128 : (t_i + 1) * 128],
                        identB[0:CR, 0:CR],
                    )
                nc.scalar.copy(qT[:, t0 : t0 + 2, :], ptp)

        def gram_row(s, j):
            st = state[s]
            qT, patt = st["qT"], st["patt"]
            for t_i in range(TPR * j, TPR * (j + 1)):
                qTs = qT[:, t_i, :]
                nc.tensor.matmul(
                    patt, qTs, qTs, start=(t_i == 0), stop=(t_i == NT - 1)
                )

        def softmax_m(s):
            st = state[s]
            patt, m = st["patt"], st["m"]
            negm = small.tile([CR, 1], F32, tag="negm", name=f"negm{s}")
            nc.vector.tensor_reduce(
                out=negm, in_=patt, axis=AX.X, op=ALU.max, negate=True
            )
            shifted = small.tile([CR, CR], F32, tag="shifted", name=f"shifted{s}")
            nc.vector.tensor_scalar(
                out=shifted, in0=patt, scalar1=negm, scalar2=-80.0,
                op0=ALU.add, op1=ALU.max,
            )
            atte = small.tile([CR, CR], F32, tag="atte", name=f"atte{s}")
            ssum = small.tile([CR, 1], F32, tag="ssum", name=f"ssum{s}")
            nc.scalar.activation(
                atte, shifted, AF.Exp, bias=0.0, scale=1.0, accum_out=ssum
            )
            rsum = small.tile([CR, 1], F32, tag="rsum", name=f"rsum{s}")
            nc.vector.reciprocal(rsum, ssum)
            attn = small.tile([CR, CR], BF16, tag="attn", name=f"attn{s}")
            nc.vector.tensor_scalar_mul(attn, atte, rsum)
            # m rows = (att.T @ w2T) = (w2 @ att).T in one 512-row matmul
            pmT = ps_mm.tile([CR, C], F32, tag="mm", name=f"pmT{s}")
            nc.tensor.matmul(pmT, attn, w2T, start=True, stop=True)
            nc.scalar.copy(m, pmT)

        def step5_chunk(s, oc):
            """y[oc] = m[oc] @ q + b2[oc] + x[oc] into fin tiles."""
            st = state[s]
            q, m = st["q"], st["m"]
            osl = slice(oc * 128, (oc + 1) * 128)
            fins = []
            for half in range(NL):
                f = fin.tile([128, LF], F32, tag="fin", name=f"fin{s}_{oc}_{half}")
                for sub in range(BPR):
                    n = half * BPR + sub
                    nsl = bass.ts(n, NF)
                    ssl = bass.ts(sub, NF)
                    p5 = ps_o.tile([128, NF], F32, tag="o5", name=f"p5{s}_{oc}_{n}")
                    nc.tensor.matmul(
                        p5, m[:, osl], q[:, nsl], start=True, stop=True
                    )
                    nc.vector.scalar_tensor_tensor(
                        out=f[:, ssl], in0=p5, scalar=b2cs[oc],
                        in1=xb[s][half][oc][:, ssl],
                        op0=ALU.add, op1=ALU.add,
                    )
                fins.append((s, oc, half, f))
            return fins

        def issue_stores(fins):
            for s, oc, half, f in fins:
                osl = slice(oc * 128, (oc + 1) * 128)
                nc.sync.dma_start(out=out_d[s, osl, bass.ts(half, LF)], in_=f)

        # ================= sample 0 stream =================
        # sync ring holds ONLY loads (all 32, completion-ordered) and then
        # stores; qT transposes ride the ACT ring right behind their evacs.
        load_x_rows(0, [2, 3])
        load_x_rows(1, [0, 1, 2, 3])
        begin_sample(0)
        stream_row(0, 0)
        tp_row(0, 0)
        stream_row(0, 1)
        tp_row(0, 1)
        gram_row(0, 0)
        stream_row(0, 2)
        tp_row(0, 2)
        gram_row(0, 1)
        stream_row(0, 3)
        tp_row(0, 3)
        gram_row(0, 2)
        gram_row(0, 3)
        softmax_m(0)

        # ========== interleave: s1 stream first-in-unit, s0 step5 ==========
        begin_sample(1)
        fins = []
        for i in range(KC):
            stream_row(1, i)
            tp_row(1, i)
            if i > 0:
                gram_row(1, i - 1)
            fins += step5_chunk(0, i)
            issue_stores(fins[-NL:])
        gram_row(1, NL - 1)
        softmax_m(1)
        for i in range(KC):
            fins1 = step5_chunk(1, i)
            issue_stores(fins1)

    nc.compile()
    return nc


_NC_CACHE = None


def _get_nc():
    global _NC_CACHE
    if _NC_CACHE is None:
        _NC_CACHE = _build_nc()
    return _NC_CACHE


def _as_f32(a):
    return np.ascontiguousarray(np.asarray(a, dtype=np.float32))


def run(inputs, trace=False):
    """Run on all 8 cores; returns (full output [B,C,W,H], BassKernelResults)."""
    nc = _get_nc()
    x = _as_f32(inputs["x"]).reshape(B, C, N)
    w1 = _as_f32(inputs["w1"])
    b1 = _as_f32(inputs["b1"])
    w2 = _as_f32(inputs["w2"])
    b2 = _as_f32(inputs["b2"])
    in_maps = [
        {
            "x": x[c * BPC : (c + 1) * BPC],
            "w1": w1,
            "b1": b1,
            "w2": w2,
            "b2": b2,
        }
        for c in range(NCORES)
    ]
    res = run_bass_kernel_spmd(nc, in_maps, list(range(NCORES)), trace=trace)
    out = np.concatenate([res.results[c]["out"] for c in range(NCORES)], axis=0)
    return out.reshape(B, C, W, H).astype(np.float32, copy=False), res


def kernel(**inputs):
    out, _ = run(inputs)
    return out
